# revision 17
# baseline (speedup 1.0000x reference)
"""Trainium2 Bass kernel for nn_Discriminator (GRU-like recurrent discriminator).

Math (per batch row):
    belta = exp(-relu(td @ Wb^T + bb))                       # (T, H)
    for t in 0..T-1:
        s = belta[t] * s
        u = sigmoid(s @ W1h^T + x[t] @ W1x^T + b1)
        r = sigmoid(s @ W2h^T + x[t] @ W2x^T + b2)
        n = tanh((r*s) @ W3h^T + x[t] @ W3x^T + b3)
        s = (1-u)*s + u*n
    out = sigmoid(s @ Wo^T + bo)

Strategy: data-parallel over 8 cores on the batch dim (B=256 -> 32/core).
Phase 1 (belta + per-gate x-contributions) is computed blockwise (16
steps/block) in fp8 DoubleRow matmuls into SBUF ring buffers; the Tile
list-scheduler interleaves this work into the recurrence's dependency
stalls.  Phase 2 (the sequential T-scan) uses bf16 weight-stationary
matmuls.

v2 changes vs v1 (817us baseline):
  * tanh-only activation set: sigmoid(z) is computed as
    0.5*(1+tanh(z/2)) with the 0.5 pre-activation scale folded into the
    host-packed weights/biases and the post-affine folded into the
    existing fused DVE ops (zero extra op count; the state-update
    dependent chain is one op SHORTER).  This removes every
    ACT_TABLE_LOAD (1.28us each, 32 of them in v1): exp and tanh
    coexist in the HW 'exp_and_others' table, sigmoid does not.
  * psr/psu double-buffered; the next step's x-contribution PSUM
    injections are emitted at the END of the step so they execute
    inside the state-update dependency stall.
  * dummy "warmer" matmuls fill the remaining PE idle gaps so the
    PE_HAM activity monitor keeps the PE array at K=8/8 (2.4 GHz)
    instead of throttling to 4/8 (1.2 GHz) after every per-step stall;
    the no-phase-1 tail (last block) gets a larger dose.
"""

import numpy as np
import ml_dtypes

B, T, IN, H = 256, 96, 512, 1024
NCORES = 8
BS = B // NCORES      # 32 batch rows per core
HC = H // 128         # 8 hidden chunks
KC = IN // 128        # 4 input chunks
CB = HC * BS          # 256 packed columns: col = chunk*BS + b
H2 = CB // 2          # 128 packed cols per half
KH = HC // 2          # 4 chunks per half

TS = 16               # time steps per phase-1 block
SC = TS * BS          # 512 psum cols per phase-1 tile
NSLOT = 2             # ring slots

NW_MID = 6            # PE warmer matmuls per step (phase-1 steps)
NW_TAIL = 36          # PE warmer matmuls per step (tail, no phase-1)

# fp8 scaling for phase-1 only (ml_dtypes.float8_e4m3: max 240)
SW = 2.0 ** 12        # x/belta weight scale
SX = 2.0 ** 5         # x scale (randn, clipped)
STD = 2.0 ** 7        # time_delta scale ([0,1))
PS1_URN = SW * SX     # 2^17: phase-1 psum scale for u/r/n jobs
PS1_B = SW * STD      # 2^19: phase-1 psum scale for belta job

BF16 = ml_dtypes.bfloat16
FP8 = ml_dtypes.float8_e4m3


def build_program(t_steps=T):
    import concourse.mybir as mybir
    import concourse.tile as tile
    from concourse import bacc
    from concourse.masks import make_identity

    f32 = mybir.dt.float32
    bf16 = mybir.dt.bfloat16
    f8 = mybir.dt.float8e4
    AF = mybir.ActivationFunctionType
    ALU = mybir.AluOpType
    DR = mybir.MatmulPerfMode.DoubleRow
    TB = t_steps * BS
    NS = t_steps // TS    # number of phase-1 blocks

    nc = bacc.Bacc("TRN2", target_bir_lowering=False)

    # ---- DRAM I/O (per core; weights replicated by the host) ----
    xt = nc.dram_tensor("xt", [KC, 128, TB], f8, kind="ExternalInput")
    tdt = nc.dram_tensor("tdt", [KC, 128, TB], bf16, kind="ExternalInput")
    # recurrent weights, bf16: [p, k, m*128+j] = W[m*128+j, k*128+p]
    # (full scale; the tanh-half-angle 0.5 rides in the half-belta ring)
    w1h = nc.dram_tensor("w1h", [128, HC, H], bf16, kind="ExternalInput")
    w2h = nc.dram_tensor("w2h", [128, HC, H], bf16, kind="ExternalInput")
    w3h = nc.dram_tensor("w3h", [128, HC, H], bf16, kind="ExternalInput")
    # x weights, fp8 DoubleRow layout (KC/2 = 2 double-chunks)
    w1x = nc.dram_tensor("w1x", [128, 2, KC // 2, H], f8, kind="ExternalInput")
    w2x = nc.dram_tensor("w2x", [128, 2, KC // 2, H], f8, kind="ExternalInput")
    w3x = nc.dram_tensor("w3x", [128, 2, KC // 2, H], f8, kind="ExternalInput")
    wbt = nc.dram_tensor("wbt", [128, KC, H], bf16, kind="ExternalInput")
    # biases: b1t/b2t = b/2 (tanh trick), b3 true; bbs = bb (pre-exp relu)
    b1t = nc.dram_tensor("b1t", [128, HC], f32, kind="ExternalInput")
    b2t = nc.dram_tensor("b2t", [128, HC], f32, kind="ExternalInput")
    b3t = nc.dram_tensor("b3t", [128, HC], f32, kind="ExternalInput")
    bbs = nc.dram_tensor("bbs", [128, HC], f32, kind="ExternalInput")
    wot = nc.dram_tensor("wot", [128, HC], f32, kind="ExternalInput")
    bot = nc.dram_tensor("bot", [1, 1], f32, kind="ExternalInput")
    out = nc.dram_tensor("out", [BS, 1], f32, kind="ExternalOutput")

    with tile.TileContext(nc) as tc:
        with (
            tc.tile_pool(name="singles", bufs=1) as S,
            tc.tile_pool(name="scp", bufs=2) as scp,
            tc.tile_pool(name="ps2", bufs=1, space="PSUM") as ps2,
            tc.tile_pool(name="ps1p", bufs=2, space="PSUM") as ps1p,
            tc.tile_pool(name="pswm", bufs=1, space="PSUM") as pswm,
        ):
            # ---- persistent SBUF ----
            sw1h = S.tile([128, HC, H], bf16)
            sw2h = S.tile([128, HC, H], bf16)
            sw3h = S.tile([128, HC, H], bf16)
            sw1x = S.tile([128, 2, KC // 2, H], f8)
            sw2x = S.tile([128, 2, KC // 2, H], f8)
            sw3x = S.tile([128, 2, KC // 2, H], f8)
            swbt = S.tile([128, KC, H], bf16)
            sb1 = S.tile([128, HC], f32)
            sb2 = S.tile([128, HC], f32)
            sb3 = S.tile([128, HC], f32)
            sbbs = S.tile([128, HC], f32)
            swo = S.tile([128, HC], f32)
            sbo = S.tile([1, 1], f32)
            ident = S.tile([128, 128], bf16)
            make_identity(nc, ident)
            # per-partition bias tile holding ln(0.5) for the half-belta exp
            bln2 = S.tile([128, 1], f32)
            nc.vector.memset(bln2, -0.6931471805599453)
            # warmer scratch (rhs for HAM-warming dummy matmuls)
            wsrc = S.tile([128, BS], bf16)
            nc.vector.memset(wsrc, 0.0)

            # rings: gate x-contributions (+bias, true scale) + belta
            ring_u = S.tile([128, NSLOT, TS, CB], bf16)
            ring_r = S.tile([128, NSLOT, TS, CB], bf16)
            ring_n = S.tile([128, NSLOT, TS, CB], bf16)
            ring_b = S.tile([128, NSLOT, TS, CB], f32)
            # x/td stream rings (fp8, scaled)
            xr = S.tile([128, NSLOT, KC, SC], f8)
            tdr = S.tile([128, NSLOT, KC, SC], bf16)

            # state: f32 carry + f32 half-decayed state (0.5 * belta * s)
            st = S.tile([128, CB], f32)
            nc.vector.memset(st, 0.0)
            stm = S.tile([128, CB], f32)
            nc.gpsimd.memset(stm, 0.0)

            # ---- upfront DMAs (phase-1 block-0 inputs first so the PE
            # starts ~5us in instead of waiting behind 6MB of wh weights) ----
            nc.sync.dma_start(out=sbbs, in_=bbs[:, :])
            nc.sync.dma_start(out=swbt, in_=wbt[:, :, :])

            # ---- phase-1 machinery (fp8 DoubleRow) ----
            def block_dmas(s):
                sl = s % NSLOT
                for k in range(KC):
                    nc.sync.dma_start(
                        out=tdr[:, sl, k, :], in_=tdt[k, :, s * SC:(s + 1) * SC]
                    )
                    nc.sync.dma_start(
                        out=xr[:, sl, k, :], in_=xt[k, :, s * SC:(s + 1) * SC]
                    )

            def emit_unit(s, jobi, m):
                """One m-chunk of one job of block s: 2 DR matmuls + post."""
                sl = s % NSLOT
                ps = ps1p.tile([128, SC], f32, tag="ps1", name="ps1")
                if jobi == 0:
                    # belta job in bf16 (accuracy); relu on DVE keeps the
                    # ACT function table at {Exp, Tanh}
                    for k in range(KC):
                        nc.tensor.matmul(
                            ps,
                            swbt[:, k, m * 128:(m + 1) * 128],
                            tdr[:, sl, k, :],
                            start=(k == 0), stop=(k == KC - 1),
                        )
                    tmp = scp.tile([128, SC], f32, tag="p1b", name="p1b")
                    nc.vector.tensor_scalar(
                        tmp, ps, sbbs[:, m:m + 1], 0.0,
                        op0=ALU.add, op1=ALU.max,
                    )
                    t3 = tmp.rearrange("p (t b) -> p t b", b=BS)
                    # ring_b stores HALF-belta: exp(-z + ln 0.5) = 0.5*e^-z.
                    # sbb = st*rb then carries s~/2, so the gate weights stay
                    # at full scale (the x2 cancels the tanh-trick x0.5) and
                    # stm = st*rb is exactly the 0.5*s~ the state mix needs.
                    nc.scalar.activation(
                        ring_b[:, sl, :, m * BS:(m + 1) * BS], t3, AF.Exp,
                        scale=-1.0, bias=bln2[:, 0:1],
                    )
                    return
                wsb, rin = (None, (sw1x, xr), (sw2x, xr), (sw3x, xr))[jobi]
                for c2 in range(KC // 2):
                    nc.tensor.matmul(
                        ps,
                        wsb[:, :, c2, m * 128:(m + 1) * 128],
                        rin[:, sl, 2 * c2:2 * c2 + 2, :],
                        start=(c2 == 0), stop=(c2 == KC // 2 - 1),
                        perf_mode=DR,
                    )
                ps3 = ps.rearrange("p (t b) -> p t b", b=BS)
                if True:
                    bias = (None, sb1, sb2, sb3)[jobi]
                    # u/r rings hold HALF the pre-activation (tanh trick);
                    # n ring holds the true pre-activation.
                    pscale = 1.0 / (2.0 * PS1_URN) if jobi < 3 else 1.0 / PS1_URN
                    oview = (None, ring_u, ring_r, ring_n)[jobi][
                        :, sl, :, m * BS:(m + 1) * BS
                    ]
                    if jobi < 3:
                        # u/r posts on the Scalar engine (Identity with
                        # per-partition bias) to keep DVE off the
                        # recurrence critical path.
                        nc.scalar.activation(
                            oview, ps3, AF.Identity,
                            bias=bias[:, m:m + 1], scale=pscale,
                        )
                    else:
                        nc.vector.tensor_scalar(
                            oview, ps3, pscale, bias[:, m:m + 1],
                            op0=ALU.mult, op1=ALU.add,
                        )

            def feed_block_units(s, lo, hi):
                """Emit units [lo, hi) of block s (unit = jobi*HC + m),
                belta job first so next-block decay factors are ready."""
                for ui in range(lo, hi):
                    emit_unit(s, ui // HC, ui % HC)

            def emit_warmers(n):
                """Dummy matmuls with no data deps: keep the PE array busy
                through dependency stalls so PE_HAM stays at K=8/8."""
                for _ in range(n):
                    wp = pswm.tile([128, BS], f32, tag="warm", name="warm")
                    nc.tensor.matmul(wp, ident, wsrc, start=True, stop=True)

            def inject(t):
                """Start the step-t gate PSUM banks with the phase-1
                x-contributions (identity matmuls).  Bank layout (8 banks):
                ps1 x2, psr, psu, psn x2 (double-buffered: the t+1 inject
                overlaps the t tanh read), warm/pso, 1 spare.  No two
                concurrently-accessed tiles share a bank (PE-W + engine-R
                on one bank is fatal)."""
                sl = (t // TS) % NSLOT
                tt = t % TS
                psr = ps2.tile([128, CB], f32, tag="psr", name="psr")
                psu = ps2.tile([128, CB], f32, tag="psu", name="psu")
                psn = ps2.tile([128, CB], f32, tag="psn", name="psn", bufs=2)
                nc.tensor.matmul(psr, ident, ring_r[:, sl, tt, :],
                                 start=True, stop=False)
                nc.tensor.matmul(psu, ident, ring_u[:, sl, tt, :],
                                 start=True, stop=False)
                nc.tensor.matmul(psn, ident, ring_n[:, sl, tt, :],
                                 start=True, stop=False)
                return psr, psu, psn

            # ---- prologue: block 0 ----
            block_dmas(0)
            nc.sync.dma_start(out=sb1, in_=b1t[:, :])
            nc.sync.dma_start(out=sb2, in_=b2t[:, :])
            nc.sync.dma_start(out=sb3, in_=b3t[:, :])
            nc.sync.dma_start(out=sw1x, in_=w1x[:, :, :, :])
            nc.sync.dma_start(out=sw2x, in_=w2x[:, :, :, :])
            nc.sync.dma_start(out=sw3x, in_=w3x[:, :, :, :])
            nc.sync.dma_start(out=sw1h, in_=w1h[:, :, :])
            nc.sync.dma_start(out=sw2h, in_=w2h[:, :, :])
            nc.sync.dma_start(out=sw3h, in_=w3h[:, :, :])
            nc.sync.dma_start(out=swo, in_=wot[:, :])
            nc.sync.dma_start(out=sbo, in_=bot[:, :])
            feed_block_units(0, 0, 4 * HC)

            # ---- recurrence ----
            sbb = S.tile([128, HC, BS], bf16)    # belta * state (matmul rhs)
            nc.vector.memset(sbb, 0.0)
            sbbf = sbb.rearrange("p c b -> p (c b)")

            pend = inject(0)

            for t in range(t_steps):
                sl = (t // TS) % NSLOT
                tt = t % TS
                psr, psu, psn = pend

                # r gate: k-outer so the low state half unblocks it
                for k in range(HC):
                    for m in range(HC):
                        nc.tensor.matmul(
                            psr[:, m * BS:(m + 1) * BS],
                            sw2h[:, k, m * 128:(m + 1) * 128],
                            sbb[:, k, :],
                            start=False,
                            stop=(k == HC - 1 and m == HC - 1),
                        )
                rg = scp.tile([128, CB], bf16, tag="rg", name="rg")
                nc.scalar.activation(rg, psr, AF.Tanh)
                # q = (1 + g_r) * sbb   (the 0.5 is folded into w3h)
                rs = scp.tile([128, HC, BS], bf16, tag="rs", name="rs")
                rsf = rs.rearrange("p c b -> p (c b)")
                nc.vector.scalar_tensor_tensor(
                    rsf, rg, 1.0, sbbf, op0=ALU.add, op1=ALU.mult
                )

                # u gate
                for k in range(HC):
                    for m in range(HC):
                        nc.tensor.matmul(
                            psu[:, m * BS:(m + 1) * BS],
                            sw1h[:, k, m * 128:(m + 1) * 128],
                            sbb[:, k, :],
                            start=False,
                            stop=(k == HC - 1 and m == HC - 1),
                        )
                # n gate (rhs = q)
                for k in range(HC):
                    for m in range(HC):
                        nc.tensor.matmul(
                            psn[:, m * BS:(m + 1) * BS],
                            sw3h[:, k, m * 128:(m + 1) * 128],
                            rs[:, k, :],
                            start=False,
                            stop=(k == HC - 1 and m == HC - 1),
                        )

                last = t == t_steps - 1
                if not last:
                    t1 = t + 1
                    rb = ring_b[:, (t1 // TS) % NSLOT, t1 % TS, :]

                ug = scp.tile([128, CB], bf16, tag="ug", name="ug")
                nc.scalar.activation(ug, psu, AF.Tanh)
                # w2x = (g_u - 1) * (0.5 * belta * s)  [off-critical]
                # (stm was computed at the end of step t-1 and equals
                #  0.5*belta(t)*s(t-1) — exactly the mix operand; note the
                #  v1 baseline consumed a one-step-shifted belta here)
                wneg = scp.tile([128, CB], f32, tag="wn", name="wneg")
                nc.vector.scalar_tensor_tensor(
                    wneg, ug, 1.0, stm, op0=ALU.subtract, op1=ALU.mult
                )
                ng = scp.tile([128, CB], bf16, tag="ng", name="ng")
                nc.scalar.activation(ng, psn, AF.Tanh)
                # d1 = (1 + g_u) * n ; st' = 0.5*d1 - w2x
                e = scp.tile([128, CB], bf16, tag="e", name="e")
                nc.vector.scalar_tensor_tensor(
                    e, ug, 1.0, ng, op0=ALU.add, op1=ALU.mult
                )
                nc.vector.scalar_tensor_tensor(
                    st, e, 0.5, wneg, op0=ALU.mult, op1=ALU.subtract
                )
                if not last:
                    # sbb' = stm' = st' * (belta/2)  (bf16 matmul rhs and
                    # f32 mix operand).  lo half on DVE: it gates the next
                    # step's first matmuls, and DVE just produced st (no
                    # sem hop).  hi half + stm on gpsimd: slack before r's
                    # k>=4 matmuls / the mid-step wneg.
                    nc.vector.tensor_mul(sbbf[:, :H2], st[:, :H2], rb[:, :H2])
                    nc.gpsimd.tensor_mul(sbbf[:, H2:], st[:, H2:], rb[:, H2:])
                    nc.gpsimd.tensor_mul(stm, st, rb)

                # ---- fill the state-update stall: next step's PSUM
                # injections, then phase-1 feed, then HAM warmers ----
                if not last:
                    pend = inject(t + 1)

                fb = t // TS + 1
                nwarm = NW_TAIL
                if fb < NS:
                    nwarm = NW_MID
                    ph = t % TS
                    if ph == 0:
                        block_dmas(fb)
                    if ph < 8:
                        ulo, uhi = 8 + 3 * ph, 8 + 3 * (ph + 1)
                    elif ph < 12:
                        ulo, uhi = 2 * (ph - 8), 2 * (ph - 8) + 2
                    else:
                        ulo = uhi = 0
                    with tc.high_priority(offset=-450):
                        for ui in range(ulo, uhi):
                            emit_unit(fb, ui // HC, ui % HC)
                if not last:
                    emit_warmers(nwarm)

            # ---- head: out = 0.5 + 0.5*tanh(s @ (Wo/2)^T + bo/2) ----
            # (the head PSUM shares the warmer bank; the head matmuls are
            #  emitted after the last warmer on the same engine, so no
            #  concurrent access)
            pso = pswm.tile([128, BS], f32, tag="warm", name="pso")[0:1, :]
            st3 = st.rearrange("p (c b) -> p c b", b=BS)
            for k in range(HC):
                nc.tensor.matmul(
                    pso, swo[:, k:k + 1], st3[:, k, :],
                    start=(k == 0), stop=(k == HC - 1),
                )
            ob = scp.tile([1, BS], f32, tag="ob", name="ob")
            nc.scalar.activation(ob, pso, AF.Tanh, bias=sbo[0:1, 0:1])
            ob2 = scp.tile([1, BS], f32, tag="ob2", name="ob2")
            nc.vector.tensor_scalar(
                ob2, ob, 0.5, 0.5, op0=ALU.mult, op1=ALU.add
            )
            nc.sync.dma_start(out=out[:, :], in_=ob2)

    nc.finalize()
    return nc


def _f8(a, clip=224.0):
    return np.clip(a, -clip, clip).astype(FP8)


def _pack_wh(w):  # [H, H] -> [128, HC, H];  out[p,k,m*128+j] = w[m*128+j,k*128+p]
    return np.ascontiguousarray(
        w.reshape(HC, 128, HC, 128).transpose(3, 2, 0, 1).reshape(128, HC, H)
    ).astype(BF16)


def _pack_wx8(w):
    # [H, IN] -> [128, 2, KC//2, H]; out[p,d,c2,m*128+j] = w[m*128+j,(2c2+d)*128+p]*SW
    t = (w * SW).reshape(HC, 128, KC // 2, 2, 128).transpose(4, 3, 2, 0, 1)
    return _f8(np.ascontiguousarray(t.reshape(128, 2, KC // 2, H)))


def _pack_wx(w):  # [H, IN] -> [128, KC, H] bf16
    return np.ascontiguousarray(
        w.reshape(HC, 128, KC, 128).transpose(3, 2, 0, 1).reshape(128, KC, H)
    ).astype(BF16)


def _pack_x(xs, t_steps):  # [BS, t, IN] -> [KC, 128, t*BS] bf16
    return np.ascontiguousarray(
        xs.reshape(BS, t_steps, KC, 128).transpose(2, 3, 1, 0).reshape(KC, 128, -1)
    ).astype(BF16)


def _pack_bias(b, scale=1.0):  # [H] -> [128, HC]
    return np.ascontiguousarray((b * scale).reshape(HC, 128).T).astype(np.float32)


def _pack_x8(xs, t_steps, scale):  # [BS, t, IN] -> [KC, 128, t*BS]
    t = (xs * scale).reshape(BS, t_steps, KC, 128).transpose(2, 3, 1, 0)
    return _f8(np.ascontiguousarray(t.reshape(KC, 128, -1)))


def prepare_in_maps(x, time_delta, Wb, bb, W1, b1, W2, b2, W3, b3, Wo, bo,
                    t_steps=T, ncores=NCORES):
    x = np.asarray(x, np.float32)
    time_delta = np.asarray(time_delta, np.float32)
    common = {
        # tanh half-angle trick: the ring holds half-belta, so sbb carries
        # s~/2 and the state-weights stay at FULL scale; only the x-side
        # contributions (b1t/b2t and the u/r ring posts) are halved.
        "w1h": _pack_wh(np.asarray(W1, np.float32)[:, :H]),
        "w2h": _pack_wh(np.asarray(W2, np.float32)[:, :H]),
        "w3h": _pack_wh(np.asarray(W3, np.float32)[:, :H]),
        "w1x": _pack_wx8(np.asarray(W1, np.float32)[:, H:]),
        "w2x": _pack_wx8(np.asarray(W2, np.float32)[:, H:]),
        "w3x": _pack_wx8(np.asarray(W3, np.float32)[:, H:]),
        "wbt": _pack_wx(np.asarray(Wb, np.float32)),
        "b1t": _pack_bias(np.asarray(b1, np.float32), 0.5),
        "b2t": _pack_bias(np.asarray(b2, np.float32), 0.5),
        "b3t": _pack_bias(np.asarray(b3, np.float32)),
        "bbs": _pack_bias(np.asarray(bb, np.float32)),
        "wot": _pack_bias(np.asarray(Wo, np.float32).reshape(H) * 0.5),
        "bot": np.asarray(bo, np.float32).reshape(1, 1) * 0.5,
    }
    in_maps = []
    for i in range(ncores):
        sl = slice(i * BS, (i + 1) * BS)
        m = dict(common)
        m["xt"] = _pack_x8(x[sl], t_steps, SX)
        m["tdt"] = _pack_x(time_delta[sl], t_steps)
        in_maps.append(m)
    return in_maps


def run(inputs, trace=False, trace_kwargs=None):
    from concourse.bass_utils import run_bass_kernel_spmd

    nc = build_program()
    in_maps = prepare_in_maps(**inputs)
    res = run_bass_kernel_spmd(
        nc, in_maps, list(range(NCORES)), trace=trace,
        trace_kwargs=trace_kwargs or {},
    )
    outs = np.concatenate(
        [np.asarray(res.results[i]["out"]) for i in range(NCORES)], axis=0
    ).astype(np.float32)
    return outs, res


def kernel(**inputs):
    outs, _ = run(inputs, trace=False)
    return outs


# revision 23
# speedup vs baseline: 1.2352x; 1.2352x over previous
"""Trainium2 Bass kernel for nn_Discriminator (GRU-like recurrent discriminator).

Math (per batch row):
    belta = exp(-relu(td @ Wb^T + bb))                       # (T, H)
    for t in 0..T-1:
        s = belta[t] * s
        u = sigmoid(s @ W1h^T + x[t] @ W1x^T + b1)
        r = sigmoid(s @ W2h^T + x[t] @ W2x^T + b2)
        n = tanh((r*s) @ W3h^T + x[t] @ W3x^T + b3)
        s = (1-u)*s + u*n
    out = sigmoid(s @ Wo^T + bo)

Strategy: data-parallel over 8 cores on the batch dim (B=256 -> 32/core).
Phase 1 (belta + per-gate x-contributions) is computed blockwise (16
steps/block) in fp8 DoubleRow matmuls into SBUF ring buffers; the Tile
list-scheduler interleaves this work into the recurrence's dependency
stalls.  Phase 2 (the sequential T-scan) uses bf16 weight-stationary
matmuls.

v2 changes vs v1 (817us baseline):
  * tanh-only activation set: sigmoid(z) is computed as
    0.5*(1+tanh(z/2)) with the 0.5 pre-activation scale folded into the
    host-packed weights/biases and the post-affine folded into the
    existing fused DVE ops (zero extra op count; the state-update
    dependent chain is one op SHORTER).  This removes every
    ACT_TABLE_LOAD (1.28us each, 32 of them in v1): exp and tanh
    coexist in the HW 'exp_and_others' table, sigmoid does not.
  * psr/psu double-buffered; the next step's x-contribution PSUM
    injections are emitted at the END of the step so they execute
    inside the state-update dependency stall.
  * dummy "warmer" matmuls fill the remaining PE idle gaps so the
    PE_HAM activity monitor keeps the PE array at K=8/8 (2.4 GHz)
    instead of throttling to 4/8 (1.2 GHz) after every per-step stall;
    the no-phase-1 tail (last block) gets a larger dose.
"""

import numpy as np
import ml_dtypes

B, T, IN, H = 256, 96, 512, 1024
NCORES = 8
BS = B // NCORES      # 32 batch rows per core
HC = H // 128         # 8 hidden chunks
KC = IN // 128        # 4 input chunks
CB = HC * BS          # 256 packed columns: col = chunk*BS + b
H2 = CB // 2          # 128 packed cols per half
KH = HC // 2          # 4 chunks per half

TS = 16               # time steps per phase-1 block
SC = TS * BS          # 512 psum cols per phase-1 tile
NSLOT = 2             # ring slots

NW_MID = 4            # PE warmer matmuls per step (phase-1 steps)
NW_TAIL = 24          # PE warmer matmuls per step (tail, no phase-1)

# fp8 scaling for phase-1 only (ml_dtypes.float8_e4m3: max 240)
SW = 2.0 ** 12        # x/belta weight scale
SX = 2.0 ** 5         # x scale (randn, clipped)
STD = 2.0 ** 7        # time_delta scale ([0,1))
PS1_URN = SW * SX     # 2^17: phase-1 psum scale for u/r/n jobs
PS1_B = SW * STD      # 2^19: phase-1 psum scale for belta job

BF16 = ml_dtypes.bfloat16
FP8 = ml_dtypes.float8_e4m3


def build_program(t_steps=T):
    import concourse.mybir as mybir
    import concourse.tile as tile
    from concourse import bacc
    from concourse.masks import make_identity

    f32 = mybir.dt.float32
    bf16 = mybir.dt.bfloat16
    f8 = mybir.dt.float8e4
    AF = mybir.ActivationFunctionType
    ALU = mybir.AluOpType
    DR = mybir.MatmulPerfMode.DoubleRow
    TB = t_steps * BS
    NS = t_steps // TS    # number of phase-1 blocks

    nc = bacc.Bacc("TRN2", target_bir_lowering=False)

    # ---- DRAM I/O (per core; weights replicated by the host) ----
    xt = nc.dram_tensor("xt", [KC, 128, TB], f8, kind="ExternalInput")
    tdt = nc.dram_tensor("tdt", [KC, 128, TB], bf16, kind="ExternalInput")
    # recurrent weights, bf16: [p, k, m*128+j] = W[m*128+j, k*128+p]
    # (full scale; the tanh-half-angle 0.5 rides in the half-belta ring)
    w1h = nc.dram_tensor("w1h", [128, HC, H], bf16, kind="ExternalInput")
    w2h = nc.dram_tensor("w2h", [128, HC, H], bf16, kind="ExternalInput")
    w3h = nc.dram_tensor("w3h", [128, HC, H], bf16, kind="ExternalInput")
    # x weights, fp8 DoubleRow layout (KC/2 = 2 double-chunks)
    w1x = nc.dram_tensor("w1x", [128, 2, KC // 2, H], f8, kind="ExternalInput")
    w2x = nc.dram_tensor("w2x", [128, 2, KC // 2, H], f8, kind="ExternalInput")
    w3x = nc.dram_tensor("w3x", [128, 2, KC // 2, H], f8, kind="ExternalInput")
    wbt = nc.dram_tensor("wbt", [128, KC, H], bf16, kind="ExternalInput")
    # biases: b1t/b2t = b/2 (tanh trick), b3 true; bbs = bb (pre-exp relu)
    b1t = nc.dram_tensor("b1t", [128, HC], f32, kind="ExternalInput")
    b2t = nc.dram_tensor("b2t", [128, HC], f32, kind="ExternalInput")
    b3t = nc.dram_tensor("b3t", [128, HC], f32, kind="ExternalInput")
    bbs = nc.dram_tensor("bbs", [128, HC], f32, kind="ExternalInput")
    wot = nc.dram_tensor("wot", [128, HC], f32, kind="ExternalInput")
    bot = nc.dram_tensor("bot", [1, 1], f32, kind="ExternalInput")
    out = nc.dram_tensor("out", [BS, 1], f32, kind="ExternalOutput")

    with tile.TileContext(nc) as tc:
        with (
            tc.tile_pool(name="singles", bufs=1) as S,
            tc.tile_pool(name="scp", bufs=2) as scp,
            tc.tile_pool(name="ps2", bufs=1, space="PSUM") as ps2,
            tc.tile_pool(name="ps1p", bufs=2, space="PSUM") as ps1p,
            tc.tile_pool(name="pswm", bufs=1, space="PSUM") as pswm,
        ):
            # ---- persistent SBUF ----
            sw1h = S.tile([128, HC, H], bf16)
            sw2h = S.tile([128, HC, H], bf16)
            sw3h = S.tile([128, HC, H], bf16)
            sw1x = S.tile([128, 2, KC // 2, H], f8)
            sw2x = S.tile([128, 2, KC // 2, H], f8)
            sw3x = S.tile([128, 2, KC // 2, H], f8)
            swbt = S.tile([128, KC, H], bf16)
            sb1 = S.tile([128, HC], f32)
            sb2 = S.tile([128, HC], f32)
            sb3 = S.tile([128, HC], f32)
            sbbs = S.tile([128, HC], f32)
            swo = S.tile([128, HC], f32)
            sbo = S.tile([1, 1], f32)
            ident = S.tile([128, 128], bf16)
            make_identity(nc, ident)
            # per-partition bias tile holding ln(0.5) for the half-belta exp
            bln2 = S.tile([128, 1], f32)
            nc.vector.memset(bln2, -0.6931471805599453)
            # warmer scratch (rhs for HAM-warming dummy matmuls)
            wsrc = S.tile([128, BS], bf16)
            nc.vector.memset(wsrc, 0.0)

            # rings: gate x-contributions (+bias, true scale) + belta
            ring_u = S.tile([128, NSLOT, TS, CB], bf16)
            ring_r = S.tile([128, NSLOT, TS, CB], bf16)
            ring_n = S.tile([128, NSLOT, TS, CB], bf16)
            ring_b = S.tile([128, NSLOT, TS, CB], f32)
            # x/td stream rings (fp8, scaled)
            xr = S.tile([128, NSLOT, KC, SC], f8)
            tdr = S.tile([128, NSLOT, KC, SC], bf16)

            # state: f32 carry + f32 half-decayed state (0.5 * belta * s)
            st = S.tile([128, CB], f32)
            nc.vector.memset(st, 0.0)
            stm = S.tile([128, CB], f32)
            nc.gpsimd.memset(stm, 0.0)

            # ---- upfront DMAs (phase-1 block-0 inputs first so the PE
            # starts ~5us in instead of waiting behind 6MB of wh weights) ----
            nc.sync.dma_start(out=sbbs, in_=bbs[:, :])
            nc.sync.dma_start(out=swbt, in_=wbt[:, :, :])

            # ---- phase-1 machinery (fp8 DoubleRow) ----
            def block_dmas(s):
                sl = s % NSLOT
                for k in range(KC):
                    nc.sync.dma_start(
                        out=tdr[:, sl, k, :], in_=tdt[k, :, s * SC:(s + 1) * SC]
                    )
                    nc.sync.dma_start(
                        out=xr[:, sl, k, :], in_=xt[k, :, s * SC:(s + 1) * SC]
                    )

            def emit_unit(s, jobi, m):
                """One m-chunk of one job of block s: 2 DR matmuls + post."""
                sl = s % NSLOT
                ps = ps1p.tile([128, SC], f32, tag="ps1", name="ps1", bufs=2)
                if jobi == 0:
                    # belta job in bf16 (accuracy); relu on DVE keeps the
                    # ACT function table at {Exp, Tanh}
                    for k in range(KC):
                        nc.tensor.matmul(
                            ps,
                            swbt[:, k, m * 128:(m + 1) * 128],
                            tdr[:, sl, k, :],
                            start=(k == 0), stop=(k == KC - 1),
                        )
                    tmp = scp.tile([128, SC], f32, tag="p1b", name="p1b")
                    nc.vector.tensor_scalar(
                        tmp, ps, sbbs[:, m:m + 1], 0.0,
                        op0=ALU.add, op1=ALU.max,
                    )
                    t3 = tmp.rearrange("p (t b) -> p t b", b=BS)
                    # ring_b stores HALF-belta: exp(-z + ln 0.5) = 0.5*e^-z.
                    # sbb = st*rb then carries s~/2, so the gate weights stay
                    # at full scale (the x2 cancels the tanh-trick x0.5) and
                    # stm = st*rb is exactly the 0.5*s~ the state mix needs.
                    nc.scalar.activation(
                        ring_b[:, sl, :, m * BS:(m + 1) * BS], t3, AF.Exp,
                        scale=-1.0, bias=bln2[:, 0:1],
                    )
                    return
                wsb, rin = (None, (sw1x, xr), (sw2x, xr), (sw3x, xr))[jobi]
                for c2 in range(KC // 2):
                    nc.tensor.matmul(
                        ps,
                        wsb[:, :, c2, m * 128:(m + 1) * 128],
                        rin[:, sl, 2 * c2:2 * c2 + 2, :],
                        start=(c2 == 0), stop=(c2 == KC // 2 - 1),
                        perf_mode=DR,
                    )
                ps3 = ps.rearrange("p (t b) -> p t b", b=BS)
                if True:
                    bias = (None, sb1, sb2, sb3)[jobi]
                    # u/r rings hold HALF the pre-activation (tanh trick);
                    # n ring holds the true pre-activation.
                    pscale = 1.0 / (2.0 * PS1_URN) if jobi < 3 else 1.0 / PS1_URN
                    oview = (None, ring_u, ring_r, ring_n)[jobi][
                        :, sl, :, m * BS:(m + 1) * BS
                    ]
                    if jobi < 3:
                        # u/r posts on the Scalar engine (Identity with
                        # per-partition bias) to keep DVE off the
                        # recurrence critical path.
                        nc.scalar.activation(
                            oview, ps3, AF.Identity,
                            bias=bias[:, m:m + 1], scale=pscale,
                        )
                    else:
                        nc.vector.tensor_scalar(
                            oview, ps3, pscale, bias[:, m:m + 1],
                            op0=ALU.mult, op1=ALU.add,
                        )

            def feed_block_units(s, lo, hi):
                """Emit units [lo, hi) of block s (unit = jobi*HC + m),
                belta job first so next-block decay factors are ready."""
                for ui in range(lo, hi):
                    emit_unit(s, ui // HC, ui % HC)

            warm_ctr = [0]

            def emit_warmers(n):
                """Dummy matmuls with no data deps: keep the PE array busy
                through dependency stalls so PE_HAM stays at K=8/8.
                Rotate over 16 disjoint 32-col slices of the warm bank so
                consecutive warmers have no PSUM WAW chain (a same-slice
                WAW serializes on the ~128-cycle drain)."""
                wp = pswm.tile([128, 16, BS], f32, tag="warm", name="warm")
                for _ in range(n):
                    i = warm_ctr[0] % 16
                    warm_ctr[0] += 1
                    nc.tensor.matmul(wp[:, i, :], ident, wsrc,
                                     start=True, stop=True)

            def inject(t):
                """Start the step-t gate PSUM banks with the phase-1
                x-contributions (identity matmuls).  Bank layout (8 banks):
                ps1 x2, psr, psu, psn_lo, psn_hi, warm/pso, 1 spare.  All
                single-buffered: each inject is emitted in the step-(t-1)
                stall and its WAR edge on the step-(t-1) tanh read is
                already satisfied by then.  No two concurrently-accessed
                tiles share a bank (PE-W + engine-R on one bank is fatal)."""
                sl = (t // TS) % NSLOT
                tt = t % TS
                psr = ps2.tile([128, CB], f32, tag="psr", name="psr")
                psu = ps2.tile([128, CB], f32, tag="psu", name="psu")
                psn_lo = ps2.tile([128, H2], f32, tag="psn_lo", name="psn_lo")
                psn_hi = ps2.tile([128, H2], f32, tag="psn_hi", name="psn_hi")
                nc.tensor.matmul(psr, ident, ring_r[:, sl, tt, :],
                                 start=True, stop=False)
                nc.tensor.matmul(psu, ident, ring_u[:, sl, tt, :],
                                 start=True, stop=False)
                nc.tensor.matmul(psn_lo, ident, ring_n[:, sl, tt, :H2],
                                 start=True, stop=False)
                nc.tensor.matmul(psn_hi, ident, ring_n[:, sl, tt, H2:],
                                 start=True, stop=False)
                return psr, psu, psn_lo, psn_hi

            # ---- prologue: block 0 ----
            block_dmas(0)
            nc.sync.dma_start(out=sb1, in_=b1t[:, :])
            nc.sync.dma_start(out=sb2, in_=b2t[:, :])
            nc.sync.dma_start(out=sb3, in_=b3t[:, :])
            nc.sync.dma_start(out=sw1x, in_=w1x[:, :, :, :])
            nc.sync.dma_start(out=sw2x, in_=w2x[:, :, :, :])
            nc.sync.dma_start(out=sw3x, in_=w3x[:, :, :, :])
            nc.sync.dma_start(out=sw1h, in_=w1h[:, :, :])
            nc.sync.dma_start(out=sw2h, in_=w2h[:, :, :])
            nc.sync.dma_start(out=sw3h, in_=w3h[:, :, :])
            nc.sync.dma_start(out=swo, in_=wot[:, :])
            nc.sync.dma_start(out=sbo, in_=bot[:, :])
            feed_block_units(0, 0, 4 * HC)

            # ---- recurrence ----
            sbb = S.tile([128, HC, BS], bf16)    # belta * state (matmul rhs)
            nc.vector.memset(sbb, 0.0)
            sbbf = sbb.rearrange("p c b -> p (c b)")

            pend = inject(0)

            for t in range(t_steps):
                sl = (t // TS) % NSLOT
                tt = t % TS
                psr, psu, psn_lo, psn_hi = pend

                # r gate: k-outer so the low state half unblocks it
                for k in range(HC):
                    for m in range(HC):
                        nc.tensor.matmul(
                            psr[:, m * BS:(m + 1) * BS],
                            sw2h[:, k, m * 128:(m + 1) * 128],
                            sbb[:, k, :],
                            start=False,
                            stop=(k == HC - 1 and m == HC - 1),
                        )
                rg = scp.tile([128, CB], bf16, tag="rg", name="rg")
                nc.scalar.activation(rg, psr, AF.Tanh)
                # q = (1 + g_r) * sbb  (sbb is s~/2, so q = sigmoid(zr)*s~)
                rs = scp.tile([128, HC, BS], bf16, tag="rs", name="rs")
                rsf = rs.rearrange("p c b -> p (c b)")
                nc.vector.scalar_tensor_tensor(
                    rsf, rg, 1.0, sbbf, op0=ALU.add, op1=ALU.mult
                )

                # u gate
                for k in range(HC):
                    for m in range(HC):
                        nc.tensor.matmul(
                            psu[:, m * BS:(m + 1) * BS],
                            sw1h[:, k, m * 128:(m + 1) * 128],
                            sbb[:, k, :],
                            start=False,
                            stop=(k == HC - 1 and m == HC - 1),
                        )
                # n gate (rhs = q), lo output half first: its tanh/state
                # chain runs under the hi half's matmuls
                for mg in range(2):
                    for k in range(HC):
                        for m in range(mg * KH, mg * KH + KH):
                            nc.tensor.matmul(
                                (psn_lo, psn_hi)[mg][
                                    :, (m - mg * KH) * BS:(m - mg * KH + 1) * BS
                                ],
                                sw3h[:, k, m * 128:(m + 1) * 128],
                                rs[:, k, :],
                                start=False,
                                stop=(k == HC - 1 and m == mg * KH + KH - 1),
                            )

                last = t == t_steps - 1
                if not last:
                    t1 = t + 1
                    rb = ring_b[:, (t1 // TS) % NSLOT, t1 % TS, :]

                ug = scp.tile([128, CB], bf16, tag="ug", name="ug")
                nc.scalar.activation(ug, psu, AF.Tanh)
                for half in range(2):
                    lo, hi = half * H2, (half + 1) * H2
                    psn = (psn_lo, psn_hi)[half]
                    sth = st[:, lo:hi]
                    ugh = ug[:, lo:hi]
                    # w2x = (g_u - 1) * (0.5*belta*s)  [off-critical]
                    # (stm was computed at the end of step t-1 and equals
                    #  0.5*belta(t)*s(t-1) — exactly the mix operand; the
                    #  v1 baseline consumed a one-step-shifted belta here)
                    wneg = scp.tile([128, H2], f32, tag=f"wn{half}",
                                    name="wneg")
                    nc.vector.scalar_tensor_tensor(
                        wneg, ugh, 1.0, stm[:, lo:hi],
                        op0=ALU.subtract, op1=ALU.mult
                    )
                    ng = scp.tile([128, H2], bf16, tag=f"ng{half}", name="ng")
                    nc.scalar.activation(ng, psn, AF.Tanh)
                    # d1 = (1 + g_u) * n ; st' = 0.5*d1 - w2x
                    e = scp.tile([128, H2], bf16, tag=f"e{half}", name="e")
                    nc.vector.scalar_tensor_tensor(
                        e, ugh, 1.0, ng, op0=ALU.add, op1=ALU.mult
                    )
                    nc.vector.scalar_tensor_tensor(
                        sth, e, 0.5, wneg, op0=ALU.mult, op1=ALU.subtract
                    )
                    if not last:
                        # sbb' = stm' = st' * (belta/2)  (bf16 matmul rhs
                        # and f32 mix operand).  lo halves on DVE (it just
                        # produced st, no sem hop; sbb-lo gates the next
                        # step's first matmuls).  hi halves on gpsimd.
                        seng = nc.vector if half == 0 else nc.gpsimd
                        seng.tensor_mul(sbbf[:, lo:hi], sth, rb[:, lo:hi])
                        nc.gpsimd.tensor_mul(stm[:, lo:hi], sth, rb[:, lo:hi])

                # ---- fill the state-update stall: next step's PSUM
                # injections, then phase-1 feed, then HAM warmers ----
                if not last:
                    pend = inject(t + 1)

                fb = t // TS + 1
                nwarm = NW_TAIL
                if fb < NS:
                    nwarm = NW_MID
                    ph = t % TS
                    if ph == 0:
                        block_dmas(fb)
                    if ph < 8:
                        ulo, uhi = 8 + 3 * ph, 8 + 3 * (ph + 1)
                    elif ph < 12:
                        ulo, uhi = 2 * (ph - 8), 2 * (ph - 8) + 2
                    else:
                        ulo = uhi = 0
                    with tc.high_priority(offset=-450):
                        for ui in range(ulo, uhi):
                            emit_unit(fb, ui // HC, ui % HC)
                if not last:
                    emit_warmers(nwarm)

            # ---- head: out = 0.5 + 0.5*tanh(s @ (Wo/2)^T + bo/2) ----
            # (the head PSUM shares the warmer bank; the head matmuls are
            #  emitted after the last warmer on the same engine, so no
            #  concurrent access)
            pso = pswm.tile([128, 16, BS], f32, tag="warm", name="pso")[0:1, 0, :]
            st3 = st.rearrange("p (c b) -> p c b", b=BS)
            for k in range(HC):
                nc.tensor.matmul(
                    pso, swo[:, k:k + 1], st3[:, k, :],
                    start=(k == 0), stop=(k == HC - 1),
                )
            ob = scp.tile([1, BS], f32, tag="ob", name="ob")
            nc.scalar.activation(ob, pso, AF.Tanh, bias=sbo[0:1, 0:1])
            ob2 = scp.tile([1, BS], f32, tag="ob2", name="ob2")
            nc.vector.tensor_scalar(
                ob2, ob, 0.5, 0.5, op0=ALU.mult, op1=ALU.add
            )
            nc.sync.dma_start(out=out[:, :], in_=ob2)

    nc.finalize()
    return nc


def _f8(a, clip=224.0):
    return np.clip(a, -clip, clip).astype(FP8)


def _pack_wh(w):  # [H, H] -> [128, HC, H];  out[p,k,m*128+j] = w[m*128+j,k*128+p]
    return np.ascontiguousarray(
        w.reshape(HC, 128, HC, 128).transpose(3, 2, 0, 1).reshape(128, HC, H)
    ).astype(BF16)


def _pack_wx8(w):
    # [H, IN] -> [128, 2, KC//2, H]; out[p,d,c2,m*128+j] = w[m*128+j,(2c2+d)*128+p]*SW
    t = (w * SW).reshape(HC, 128, KC // 2, 2, 128).transpose(4, 3, 2, 0, 1)
    return _f8(np.ascontiguousarray(t.reshape(128, 2, KC // 2, H)))


def _pack_wx(w):  # [H, IN] -> [128, KC, H] bf16
    return np.ascontiguousarray(
        w.reshape(HC, 128, KC, 128).transpose(3, 2, 0, 1).reshape(128, KC, H)
    ).astype(BF16)


def _pack_x(xs, t_steps):  # [BS, t, IN] -> [KC, 128, t*BS] bf16
    return np.ascontiguousarray(
        xs.reshape(BS, t_steps, KC, 128).transpose(2, 3, 1, 0).reshape(KC, 128, -1)
    ).astype(BF16)


def _pack_bias(b, scale=1.0):  # [H] -> [128, HC]
    return np.ascontiguousarray((b * scale).reshape(HC, 128).T).astype(np.float32)


def _pack_x8(xs, t_steps, scale):  # [BS, t, IN] -> [KC, 128, t*BS]
    t = (xs * scale).reshape(BS, t_steps, KC, 128).transpose(2, 3, 1, 0)
    return _f8(np.ascontiguousarray(t.reshape(KC, 128, -1)))


def prepare_in_maps(x, time_delta, Wb, bb, W1, b1, W2, b2, W3, b3, Wo, bo,
                    t_steps=T, ncores=NCORES):
    x = np.asarray(x, np.float32)
    time_delta = np.asarray(time_delta, np.float32)
    common = {
        # tanh half-angle trick: the ring holds half-belta, so sbb carries
        # s~/2 and the state-weights stay at FULL scale; only the x-side
        # contributions (b1t/b2t and the u/r ring posts) are halved.
        "w1h": _pack_wh(np.asarray(W1, np.float32)[:, :H]),
        "w2h": _pack_wh(np.asarray(W2, np.float32)[:, :H]),
        "w3h": _pack_wh(np.asarray(W3, np.float32)[:, :H]),
        "w1x": _pack_wx8(np.asarray(W1, np.float32)[:, H:]),
        "w2x": _pack_wx8(np.asarray(W2, np.float32)[:, H:]),
        "w3x": _pack_wx8(np.asarray(W3, np.float32)[:, H:]),
        "wbt": _pack_wx(np.asarray(Wb, np.float32)),
        "b1t": _pack_bias(np.asarray(b1, np.float32), 0.5),
        "b2t": _pack_bias(np.asarray(b2, np.float32), 0.5),
        "b3t": _pack_bias(np.asarray(b3, np.float32)),
        "bbs": _pack_bias(np.asarray(bb, np.float32)),
        "wot": _pack_bias(np.asarray(Wo, np.float32).reshape(H) * 0.5),
        "bot": np.asarray(bo, np.float32).reshape(1, 1) * 0.5,
    }
    in_maps = []
    for i in range(ncores):
        sl = slice(i * BS, (i + 1) * BS)
        m = dict(common)
        m["xt"] = _pack_x8(x[sl], t_steps, SX)
        m["tdt"] = _pack_x(time_delta[sl], t_steps)
        in_maps.append(m)
    return in_maps


def run(inputs, trace=False, trace_kwargs=None):
    from concourse.bass_utils import run_bass_kernel_spmd

    nc = build_program()
    in_maps = prepare_in_maps(**inputs)
    res = run_bass_kernel_spmd(
        nc, in_maps, list(range(NCORES)), trace=trace,
        trace_kwargs=trace_kwargs or {},
    )
    outs = np.concatenate(
        [np.asarray(res.results[i]["out"]) for i in range(NCORES)], axis=0
    ).astype(np.float32)
    return outs, res


def kernel(**inputs):
    outs, _ = run(inputs, trace=False)
    return outs
